# revision 2
# baseline (speedup 1.0000x reference)
"""Trainium2 Bass kernel for nn_NeuralNet_19516331393457 (dense_mlp).

Pipeline: x = embed[data] (48-entry table); h1 = relu(x@W1+b1);
h2 = tanh(h1@W2+b2); out = h2@W3+b3; return out[argmax(F(out0, out1))].

Strategy (data-parallel over N=500000 on 8 cores):
  - Host: tiny-table gather embed[data] fused with a tile-blocked transpose
    so each device chunk is a contiguous [128 feat, 512 samples] tile.
  - Device (per core, 62976 padded samples = 123 chunks of 512), float32r
    matmuls (measured ~1 cyc/col when batched with the same stationary):
      * chunks grouped by 4 so each weight matrix stays stationary across
        4 back-to-back matmuls (LDWEIGHTS amortized)
      * MM3 [2,512] outputs pair-packed at free offsets of a [2,1024]
        two-bank PSUM tile; one pack-copy per 2 chunks into [2, 16K]
        staging pieces; 4 contiguous output DMAs per core
      * PSUM evictions (relu, tanh, pack-copy) balanced across DVE/ACT
  - Host: decode [2, NPC] outs, compute F in float64, global argmax,
    return out[argmax] + b3 (min-subtraction doesn't change argmax).
"""

import numpy as np

import concourse.mybir as mybir
import concourse.tile as tile
from concourse import bacc
from concourse.bass_utils import run_bass_kernel_spmd

N = 500000
D = 128
H1 = 128
H2 = 64
NCLS = 2
NCORES = 8
CHUNK = 512
NPC_RAW = N // NCORES              # 62500 samples per core
CHUNKS = -(-NPC_RAW // CHUNK)      # 123 chunks per core
NPC = CHUNKS * CHUNK               # 62976 padded samples per core
G = 4                              # chunks per stationary-weight group
QC = 32                            # chunks per output staging piece

_F32 = mybir.dt.float32
_F32R = mybir.dt.float32r


def _build_bass():
    nc = bacc.Bacc(
        "TRN2",
        target_bir_lowering=False,
        debug=False,
        enable_asserts=False,
        num_devices=NCORES,
    )
    # x is stored two chunks per DMA tile: [ceil(CHUNKS/2), D, 2*CHUNK]
    npairs = (CHUNKS + 1) // 2
    x_t = nc.dram_tensor("x_t", [npairs, D, 2 * CHUNK], _F32R,
                         kind="ExternalInput")
    w1 = nc.dram_tensor("w1", [D, H1], _F32R, kind="ExternalInput")
    w2 = nc.dram_tensor("w2", [H1, H2], _F32R, kind="ExternalInput")
    w3 = nc.dram_tensor("w3", [H2, NCLS], _F32R, kind="ExternalInput")
    b1 = nc.dram_tensor("b1", [H1, 1], _F32, kind="ExternalInput")
    b2 = nc.dram_tensor("b2", [H2, 1], _F32, kind="ExternalInput")
    out_d = nc.dram_tensor("out_pairs", [2, NPC], _F32, kind="ExternalOutput")

    with tile.TileContext(nc) as tc:
        with (
            tc.tile_pool(name="w", bufs=1) as wpool,
            tc.tile_pool(name="x", bufs=3) as xpool,
            tc.tile_pool(name="h1", bufs=6) as h1pool,
            tc.tile_pool(name="h2", bufs=4) as h2pool,
            tc.tile_pool(name="ob", bufs=2) as obpool,
            tc.tile_pool(name="p1", bufs=4, space="PSUM") as p1pool,
            tc.tile_pool(name="p2", bufs=2, space="PSUM") as p2pool,
            tc.tile_pool(name="po", bufs=2, space="PSUM") as popool,
        ):
            w1sb = wpool.tile([D, H1], _F32R)
            nc.sync.dma_start(w1sb[:], w1[:, :])
            w2sb = wpool.tile([H1, H2], _F32R)
            nc.sync.dma_start(w2sb[:], w2[:, :])
            w3sb = wpool.tile([H2, NCLS], _F32R)
            nc.sync.dma_start(w3sb[:], w3[:, :])
            b1sb = wpool.tile([H1, 1], _F32)
            nc.sync.dma_start(b1sb[:], b1[:, :])
            b2sb = wpool.tile([H2, 1], _F32)
            nc.sync.dma_start(b2sb[:], b2[:, :])

            xts = {}     # pair index -> sbuf tile [D, 2*CHUNK]
            h1ts = {}    # chunk -> sbuf tile [H1, CHUNK]
            h2ts = {}    # chunk -> sbuf tile [H2, CHUNK]
            pos = {}     # pair index -> psum tile [2, 2*CHUNK]
            obs = {}     # piece index -> sbuf tile [2, QC*CHUNK]
            p1s = {}

            ngroups = -(-CHUNKS // G)
            for g in range(ngroups):
                chunks = list(range(G * g, min(G * (g + 1), CHUNKS)))

                for c in chunks:
                    if c % 2 == 0:
                        xt = xpool.tile([D, 2 * CHUNK], _F32R,
                                        name=f"xt{c // 2}", tag="xt")
                        nc.sync.dma_start(xt[:], x_t[c // 2, :, :])
                        xts[c // 2] = xt

                # MM1 x4 (W1 stationary)
                for c in chunks:
                    p1 = p1pool.tile([H1, CHUNK], _F32, name=f"p1_{c}",
                                     tag="p1")
                    xsrc = xts[c // 2][:, (c % 2) * CHUNK : (c % 2 + 1) * CHUNK]
                    nc.tensor.matmul(p1[:], w1sb[:], xsrc, start=True,
                                     stop=True)
                    p1s[c] = p1
                # relu evictions: h1 = max(p1 + b1, 0); alternate DVE/ACT
                for c in chunks:
                    h1t = h1pool.tile([H1, CHUNK], _F32R, name=f"h1_{c}",
                                      tag="h1")
                    nc.vector.tensor_scalar(
                        h1t[:], p1s[c][:], b1sb[:], 0.0,
                        mybir.AluOpType.add, mybir.AluOpType.max,
                    )
                    h1ts[c] = h1t

                # MM2 x4 (W2 stationary)
                for c in chunks:
                    p2 = p2pool.tile([H2, CHUNK], _F32, name=f"p2_{c}",
                                     tag="p2")
                    nc.tensor.matmul(p2[:], w2sb[:], h1ts[c][:], start=True,
                                     stop=True)
                    h2t = h2pool.tile([H2, CHUNK], _F32R, name=f"h2_{c}",
                                      tag="h2")
                    nc.scalar.activation(
                        h2t[:], p2[:], mybir.ActivationFunctionType.Tanh,
                        bias=b2sb[:],
                    )
                    h2ts[c] = h2t

                # MM3 x4 (W3 stationary) -> [2, CHUNK] psum, packed into
                # [2, QC*CHUNK] staging pieces, 4 output DMAs per core
                for c in chunks:
                    po = popool.tile([2, CHUNK], _F32, name=f"po_{c}",
                                     tag="po")
                    nc.tensor.matmul(po[:], w3sb[:], h2ts[c][:], start=True,
                                     stop=True)
                    q = c // QC
                    if q not in obs:
                        obs[q] = obpool.tile([2, QC * CHUNK], _F32,
                                             name=f"ob{q}", tag="ob")
                    dst = obs[q][:, (c % QC) * CHUNK : (c % QC + 1) * CHUNK]
                    if c % 2 == 0:
                        nc.vector.tensor_copy(dst, po[:])
                    else:
                        nc.scalar.copy(dst, po[:])
                    if c == CHUNKS - 1 or (c % QC) == QC - 1:
                        npiece = min(QC * CHUNK, NPC - q * QC * CHUNK)
                        nc.sync.dma_start(
                            out_d[:, q * QC * CHUNK :
                                  q * QC * CHUNK + npiece],
                            obs[q][:, :npiece],
                        )

    nc.compile()
    return nc


_NC_CACHE = None


def _get_nc():
    global _NC_CACHE
    if _NC_CACHE is None:
        _NC_CACHE = _build_bass()
    return _NC_CACHE


def _F64(x, y):
    return (
        3.0 * (1.0 - x) ** 2 * np.exp(-(x**2) - (y + 1.0) ** 2)
        - 10.0 * (x / 5.0 - x**3 - y**5) * np.exp(-(x**2) - y**2)
        - 1.0 / (3.0 ** np.exp(-((x + 1.0) ** 2) - y**2))
    )


def make_in_maps(data, embed, W1, b1, W2, b2, W3, b3):
    data = np.asarray(data)
    table = np.asarray(embed, dtype=np.float32).reshape(-1)
    W1 = np.ascontiguousarray(W1, dtype=np.float32)
    W2 = np.ascontiguousarray(W2, dtype=np.float32)
    W3 = np.ascontiguousarray(W3, dtype=np.float32)
    b1c = np.ascontiguousarray(b1, dtype=np.float32).reshape(H1, 1)
    b2c = np.ascontiguousarray(b2, dtype=np.float32).reshape(H2, 1)

    npairs = (CHUNKS + 1) // 2
    in_maps = []
    for c in range(NCORES):
        dshard = data[c * NPC_RAW : (c + 1) * NPC_RAW]
        dpad = np.zeros((npairs * 2 * CHUNK, D), dtype=dshard.dtype)
        dpad[:NPC_RAW] = dshard
        # fused gather + tile-blocked transpose: [npairs, D(feat), 2*CHUNK]
        xt = np.ascontiguousarray(
            table[dpad.reshape(npairs, 2 * CHUNK, D).transpose(0, 2, 1)]
        )
        in_maps.append(
            {"x_t": xt, "w1": W1, "w2": W2, "w3": W3, "b1": b1c, "b2": b2c}
        )
    return in_maps


def kernel(data, embed, W1, b1, W2, b2, W3, b3):
    b3c = np.asarray(b3, dtype=np.float32).reshape(NCLS)
    nc = _get_nc()
    in_maps = make_in_maps(data, embed, W1, b1, W2, b2, W3, b3)

    res = run_bass_kernel_spmd(nc, in_maps, core_ids=list(range(NCORES)))

    outs = []
    for c in range(NCORES):
        op = res.results[c]["out_pairs"]  # [2, NPC]
        outs.append(op.T[:NPC_RAW])
    out_all = np.concatenate(outs, axis=0) + b3c  # [N, 2] fp32

    x64 = out_all[:, 0].astype(np.float64)
    y64 = out_all[:, 1].astype(np.float64)
    pred = _F64(x64, y64)
    idx = int(np.argmax(pred))
    return out_all[idx].astype(np.float32)

